# revision 12
# baseline (speedup 1.0000x reference)
"""Coord2HeatmapNet Trainium2 kernel.

out[b,c,j,i] = 10*exp(-(((i+.5)/128 - x)^2 + ((j+.5)/128 - y)^2) / (2*(2/128)^2))

Exploited structure:
  * Separable: each heatmap = fy[j] (x) fx[i] outer product.
  * The value at r pixels from the peak is 10*exp(-r^2/8); beyond ~7 px it is
    under the correctness gate, so only a WIN=14-row window per heatmap is
    materialized; the pre-zeroed output buffer keeps the rest 0.
  * Derivative_Erf activation = 2/sqrt(pi)*exp(-t^2): one ScalarE op per
    gaussian factor vector.
  * Heatmap h = 5p + g lives on partition p of group g, so the per-partition
    coord table is CONTIGUOUS in DRAM: one 40B run per partition, one cheap
    descriptor each -- the completion semaphore fires ~5us earlier than the
    stride-2 gather layout (h = p + 128g) it replaces.
  * Partition p of group g holds the whole 14x128 window of its heatmap as
    1792 contiguous floats. The outer product is one DVE tensor_tensor with
    stride-0 broadcasts; the write-out is ONE indirect scatter DMA per group
    (one offset per partition, 7KB contiguous per heatmap at its
    data-dependent window position).
  * The scatters write provably-disjoint blocks; Tile's conservative WAW dep
    over 'out' is cleared after each scatter so they pipeline behind the DVE.

Sharding: pure data parallel, 8 batches per core across 8 NeuronCores.
"""
import sys

for _p in ("/opt/trn_rl_repo", "/root/.axon_site", "/root/.axon_site/_ro/trn_rl_repo",
           "/root/.axon_site/_ro/pypackages"):
    if _p not in sys.path:
        sys.path.append(_p)

import numpy as np

S = 128
NUM_CLASS = 68
B_TOTAL = 64
N_CORES = 8
B_LOC = B_TOTAL // N_CORES            # 8 batches per core
NHM = B_LOC * NUM_CLASS               # 544 heatmaps per core
WIN = 14                              # window rows per heatmap
NG = 5                                # heatmap h = 5p + g; slots with
                                      # h >= NHM are dropped by the scatter's
                                      # bounds check (96 fake slots)
FREE = WIN * S                        # 1792 elems (7KB) per heatmap window
SIGMA = 2.0 / S
DENOM = 2.0 * SIGMA * SIGMA           # 1/2048
SINV = float(np.sqrt(1.0 / DENOM))    # 45.254834
A = SINV / S
AMP = float(10.0 * np.pi / 4.0)
OUT_ELEMS = NHM * S * S

_cache = {}


def _build():
    import concourse.bass as bass
    import concourse.tile as tile
    from concourse import bacc, mybir
    from concourse.bass import IndirectOffsetOnAxis
    from concourse.bass_types import AP

    f32 = mybir.dt.float32
    f16 = mybir.dt.float16
    i32 = mybir.dt.int32
    nc = bacc.Bacc("TRN2", target_bir_lowering=False, debug=False,
                   num_devices=N_CORES)

    coords = nc.dram_tensor("coords", [B_LOC, 2 * NUM_CLASS], f32,
                            kind="ExternalInput")
    out = nc.dram_tensor("out", [OUT_ELEMS], f32, kind="ExternalOutput")
    o2d = out.ap().rearrange("(a b) -> a b", b=1)
    cflat = coords.ap().rearrange("b f -> (b f)")

    derf = mybir.ActivationFunctionType.Derivative_Erf
    op = mybir.AluOpType
    JC = WIN // 2

    with tile.TileContext(nc) as tc:
        with tc.tile_pool(name="tabs", bufs=1) as tp, \
             tc.tile_pool(name="main", bufs=5) as mp, \
             tc.tile_pool(name="vecs", bufs=2) as vp:
            # ---- coord table: C[p, 2g+h] = coords_flat[10p + 2g + h], i.e.
            # (x, y) of heatmap 5p+g at cols (2g, 2g+1). One contiguous 40B
            # run per partition; partition 108 split off to stay in bounds
            # (it only owns groups 0..3 = 8 elems).
            C = tp.tile([128, 10], f32)
            nc.sync.dma_start(
                C[0:108, :],
                AP(tensor=cflat.tensor, offset=0, ap=[[10, 108], [1, 10]]))
            nc.scalar.dma_start(
                C[108:109, 0:8],
                AP(tensor=cflat.tensor, offset=1080, ap=[[8, 1], [1, 8]]))
            cp = C[:].ap[0][0]          # C partition stride
            Y5 = AP(tensor=C[:].tensor, offset=C[:].offset + 1,
                    ap=[[cp, 128], [2, 5]])

            # iotas (gpsimd) run while the coord DMAs are in flight
            IOTA_I = tp.tile([128, S], f32)
            nc.gpsimd.iota(IOTA_I[:], pattern=[[1, S]], base=0,
                           channel_multiplier=0,
                           allow_small_or_imprecise_dtypes=True)
            RIOTA = tp.tile([128, WIN], f32)
            nc.gpsimd.iota(RIOTA[:], pattern=[[1, WIN]], base=0,
                           channel_multiplier=0,
                           allow_small_or_imprecise_dtypes=True)
            # KI5[p, g] = 5p + g, scaled to the heatmap DRAM base below
            KI5 = tp.tile([128, 5], i32)
            nc.gpsimd.iota(KI5[:], pattern=[[1, 5]], base=0,
                           channel_multiplier=5,
                           allow_small_or_imprecise_dtypes=True)
            nc.vector.tensor_scalar_mul(KI5[:], KI5[:], S * S)

            # warm-ups: a dep-free activation forces both act-table loads to
            # run back-to-back during the coords DMA instead of waiting on
            # the bias chain; a tiny indirect GATHER (read-only) pays the Q7
            # SWDGE IRAM warmup before the first real scatter.
            warm = tp.tile([128, 1], f32)
            nc.scalar.activation(warm[0:1, :], IOTA_I[0:1, 0:1], derf)
            WOFF = tp.tile([2, 1], i32)
            nc.gpsimd.iota(WOFF[:], pattern=[[1, 1]], base=OUT_ELEMS,
                           channel_multiplier=0,
                           allow_small_or_imprecise_dtypes=True)
            wg = tp.tile([2, 1], f16)
            nc.gpsimd.iota(wg[:], pattern=[[1, 1]], base=0,
                           channel_multiplier=0,
                           allow_small_or_imprecise_dtypes=True)
            nc.gpsimd.indirect_dma_start(
                o2d, IndirectOffsetOnAxis(ap=WOFF[:], axis=0),
                wg[:], None,
                bounds_check=OUT_ELEMS - FREE, oob_is_err=False)
            tc.dep_state.clear_tensor_accesses("out")

            # ---- bulk table math ----
            # BXY = a/2 - s*coord: fx bias at even cols, fy bias (minus the
            # a*jo term added below) at odd cols
            BXY = tp.tile([128, 10], f32)
            nc.vector.tensor_scalar(BXY[:], C[:], -SINV, A * 0.5,
                                    op.mult, op.add)
            # JI5 = jo + WIN/2 = clamp(rint(128y), WIN/2, S - WIN/2); the
            # -WIN/2 is folded into the BY5 / OFF5 constants.
            JF5 = tp.tile([128, 5], f32)
            nc.vector.tensor_scalar_mul(JF5[:], Y5, float(S))
            JI5 = tp.tile([128, 5], i32)
            nc.vector.tensor_copy(JI5[:], JF5[:])
            nc.vector.tensor_scalar(JI5[:], JI5[:], S - JC, JC,
                                    op.min, op.max)
            JOF5 = tp.tile([128, 5], f32)
            nc.vector.tensor_copy(JOF5[:], JI5[:])
            # fy bias: a*jo + (a/2 - s*y)
            BY5 = tp.tile([128, 5], f32)
            nc.vector.tensor_scalar(BY5[:], JOF5[:], A, -A * JC,
                                    op.mult, op.add)
            BXYo = AP(tensor=BXY[:].tensor, offset=BXY[:].offset + 1,
                      ap=[[BXY[:].ap[0][0], 128], [2, 5]])
            nc.vector.tensor_add(BY5[:], BY5[:], BXYo)
            # scatter offsets: h*S*S + jo*S
            OFF5 = tp.tile([128, 5], i32)
            nc.vector.tensor_scalar(OFF5[:], JI5[:], S, S * JC,
                                    op.mult, op.subtract)
            nc.vector.tensor_add(OFF5[:], OFF5[:], KI5[:])
            # fake slots (h >= 544) need no masking: KI5 is exact iota data
            # and JI5 is int-clamped to [JC, S-JC], so their offsets are
            # always >= 544*S*S > the scatter bounds check -> dropped.

            # ---- main loop: one 128-partition group per iteration (the
            # full width keeps the scatter descriptors sprayed across all
            # 16 SDMA engines; narrower scatters collapse onto a few) ----
            for g in range(NG):
                n = 128
                FX = vp.tile([128, S], f16, tag="fx")      # fx row per hm
                nc.scalar.activation(FX[0:n, :], IOTA_I[0:n, :], derf,
                                     bias=BXY[0:n, 2 * g:2 * g + 1], scale=A)
                FY = vp.tile([128, WIN], f16, tag="fy")    # fy row per hm
                nc.scalar.activation(FY[0:n, :], RIOTA[0:n, :], derf,
                                     bias=BY5[0:n, g:g + 1], scale=A)
                nc.vector.tensor_scalar_mul(FY[0:n, :], FY[0:n, :], AMP)

                fyap = FY[0:n, :]
                fxap = FX[0:n, :]
                G = mp.tile([128, FREE], f16, tag="g")
                in0 = AP(tensor=fyap.tensor, offset=fyap.offset,
                         ap=[[fyap.ap[0][0], n], [1, WIN], [0, S]])
                in1 = AP(tensor=fxap.tensor, offset=fxap.offset,
                         ap=[[fxap.ap[0][0], n], [0, WIN], [1, S]])
                nc.vector.tensor_tensor(G[0:n, :], in0, in1, op.mult)
                nc.gpsimd.indirect_dma_start(
                    o2d,
                    IndirectOffsetOnAxis(ap=OFF5[0:n, g:g + 1], axis=0),
                    G[0:n, :], None,
                    bounds_check=OUT_ELEMS - FREE, oob_is_err=False)
                # The scatters write provably-disjoint window blocks (each
                # heatmap owns its 16384-elem range), but Tile can't see
                # that through the dynamic offsets and would serialize them
                # on a WAW dep over 'out'. Clearing the recorded accesses
                # lets the scatters pipeline; kernel-end completion is still
                # enforced through each scatter's G-tile release.
                tc.dep_state.clear_tensor_accesses("out")

    nc.compile()
    return nc


def _get_nc():
    if "nc" not in _cache:
        _cache["nc"] = _build()
    return _cache["nc"]


def _run(coords_full, trace=False):
    from concourse.bass_utils import run_bass_kernel_spmd

    coords_full = np.ascontiguousarray(np.asarray(coords_full, dtype=np.float32))
    assert coords_full.shape == (B_TOTAL, 2 * NUM_CLASS)
    nc = _get_nc()
    in_maps = [{"coords": coords_full[i * B_LOC:(i + 1) * B_LOC]}
               for i in range(N_CORES)]
    br = run_bass_kernel_spmd(nc, in_maps, core_ids=list(range(N_CORES)),
                              trace=trace)
    parts = [br.results[i]["out"].reshape(B_LOC, NUM_CLASS, S, S)
             for i in range(N_CORES)]
    full = np.concatenate(parts, axis=0)
    return full, br


def kernel(coords):
    return _run(coords, trace=False)[0]


# revision 14
# speedup vs baseline: 1.0487x; 1.0487x over previous
"""Coord2HeatmapNet Trainium2 kernel.

out[b,c,j,i] = 10*exp(-(((i+.5)/128 - x)^2 + ((j+.5)/128 - y)^2) / (2*(2/128)^2))

Exploited structure:
  * Separable: each heatmap = fy[j] (x) fx[i] outer product.
  * The value at r pixels from the peak is 10*exp(-r^2/8); beyond ~7 px it is
    under the correctness gate, so only a WIN=14-row window per heatmap is
    materialized; the pre-zeroed output buffer keeps the rest 0.
  * Derivative_Erf activation = 2/sqrt(pi)*exp(-t^2): one ScalarE op per
    gaussian factor vector.
  * Heatmap h = 5p + g lives on partition p of group g, so the per-partition
    coord table is CONTIGUOUS in DRAM: one 40B run per partition, one cheap
    descriptor each -- the completion semaphore fires ~5us earlier than the
    stride-2 gather layout (h = p + 128g) it replaces.
  * Partition p of group g holds the whole 14x128 window of its heatmap as
    1792 contiguous floats. The outer product is one DVE tensor_tensor with
    stride-0 broadcasts; the write-out is ONE indirect scatter DMA per group
    (one offset per partition, 7KB contiguous per heatmap at its
    data-dependent window position).
  * The scatters write provably-disjoint blocks; Tile's conservative WAW dep
    over 'out' is cleared after each scatter so they pipeline behind the DVE.

Sharding: pure data parallel, 8 batches per core across 8 NeuronCores.
"""
import sys

for _p in ("/opt/trn_rl_repo", "/root/.axon_site", "/root/.axon_site/_ro/trn_rl_repo",
           "/root/.axon_site/_ro/pypackages"):
    if _p not in sys.path:
        sys.path.append(_p)

import numpy as np

S = 128
NUM_CLASS = 68
B_TOTAL = 64
N_CORES = 8
B_LOC = B_TOTAL // N_CORES            # 8 batches per core
NHM = B_LOC * NUM_CLASS               # 544 heatmaps per core
WIN = 13                              # window rows per heatmap
NG = 5                                # heatmap h = 5p + g; slots with
                                      # h >= NHM are dropped by the scatter's
                                      # bounds check (96 fake slots)
FREE = WIN * S                        # 1792 elems (7KB) per heatmap window
SIGMA = 2.0 / S
DENOM = 2.0 * SIGMA * SIGMA           # 1/2048
SINV = float(np.sqrt(1.0 / DENOM))    # 45.254834
A = SINV / S
AMP = float(10.0 * np.pi / 4.0)
OUT_ELEMS = NHM * S * S

_cache = {}


def _build():
    import concourse.bass as bass
    import concourse.tile as tile
    from concourse import bacc, mybir
    from concourse.bass import IndirectOffsetOnAxis
    from concourse.bass_types import AP

    f32 = mybir.dt.float32
    f16 = mybir.dt.float16
    i32 = mybir.dt.int32
    nc = bacc.Bacc("TRN2", target_bir_lowering=False, debug=False,
                   num_devices=N_CORES)

    coords = nc.dram_tensor("coords", [B_LOC, 2 * NUM_CLASS], f32,
                            kind="ExternalInput")
    out = nc.dram_tensor("out", [OUT_ELEMS], f32, kind="ExternalOutput")
    o2d = out.ap().rearrange("(a b) -> a b", b=1)
    cflat = coords.ap().rearrange("b f -> (b f)")

    derf = mybir.ActivationFunctionType.Derivative_Erf
    op = mybir.AluOpType
    JC = WIN // 2

    with tile.TileContext(nc) as tc:
        with tc.tile_pool(name="tabs", bufs=1) as tp, \
             tc.tile_pool(name="main", bufs=5) as mp, \
             tc.tile_pool(name="vecs", bufs=2) as vp:
            # ---- coord table: C[p, 2g+h] = coords_flat[10p + 2g + h], i.e.
            # (x, y) of heatmap 5p+g at cols (2g, 2g+1). One contiguous 40B
            # run per partition; partition 108 split off to stay in bounds
            # (it only owns groups 0..3 = 8 elems).
            C = tp.tile([128, 10], f32)
            nc.sync.dma_start(
                C[0:108, :],
                AP(tensor=cflat.tensor, offset=0, ap=[[10, 108], [1, 10]]))
            nc.scalar.dma_start(
                C[108:109, 0:8],
                AP(tensor=cflat.tensor, offset=1080, ap=[[8, 1], [1, 8]]))
            cp = C[:].ap[0][0]          # C partition stride
            Y5 = AP(tensor=C[:].tensor, offset=C[:].offset + 1,
                    ap=[[cp, 128], [2, 5]])

            # iotas (gpsimd) run while the coord DMAs are in flight
            IOTA_I = tp.tile([128, S], f32)
            nc.gpsimd.iota(IOTA_I[:], pattern=[[1, S]], base=0,
                           channel_multiplier=0,
                           allow_small_or_imprecise_dtypes=True)
            RIOTA = tp.tile([128, WIN], f32)
            nc.gpsimd.iota(RIOTA[:], pattern=[[1, WIN]], base=0,
                           channel_multiplier=0,
                           allow_small_or_imprecise_dtypes=True)
            # KI5[p, g] = 5p + g, scaled to the heatmap DRAM base below
            KI5 = tp.tile([128, 5], i32)
            nc.gpsimd.iota(KI5[:], pattern=[[1, 5]], base=0,
                           channel_multiplier=5,
                           allow_small_or_imprecise_dtypes=True)
            nc.vector.tensor_scalar_mul(KI5[:], KI5[:], S * S)

            # warm-ups: a dep-free activation forces both act-table loads to
            # run back-to-back during the coords DMA instead of waiting on
            # the bias chain; a tiny indirect GATHER (read-only) pays the Q7
            # SWDGE IRAM warmup before the first real scatter.
            warm = tp.tile([128, 1], f32)
            nc.scalar.activation(warm[0:1, :], IOTA_I[0:1, 0:1], derf)
            WOFF = tp.tile([128, 1], i32)
            nc.gpsimd.iota(WOFF[:], pattern=[[1, 1]], base=OUT_ELEMS,
                           channel_multiplier=0,
                           allow_small_or_imprecise_dtypes=True)
            wg = tp.tile([128, 1], f16)
            nc.gpsimd.iota(wg[:], pattern=[[1, 1]], base=0,
                           channel_multiplier=0,
                           allow_small_or_imprecise_dtypes=True)
            nc.gpsimd.indirect_dma_start(
                o2d, IndirectOffsetOnAxis(ap=WOFF[:], axis=0),
                wg[:], None,
                bounds_check=OUT_ELEMS - FREE, oob_is_err=False)
            tc.dep_state.clear_tensor_accesses("out")

            # ---- bulk table math ----
            # BXY = a/2 - s*coord: fx bias at even cols, fy bias (minus the
            # a*jo term added below) at odd cols
            BXY = tp.tile([128, 10], f32)
            nc.vector.tensor_scalar(BXY[:], C[:], -SINV, A * 0.5,
                                    op.mult, op.add)
            # JI5 = jo + WIN/2 = clamp(rint(128y), WIN/2, S - WIN/2); the
            # -WIN/2 is folded into the BY5 / OFF5 constants.
            JF5 = tp.tile([128, 5], f32)
            nc.vector.tensor_scalar_mul(JF5[:], Y5, float(S))
            JI5 = tp.tile([128, 5], i32)
            nc.vector.tensor_copy(JI5[:], JF5[:])
            nc.vector.tensor_scalar(JI5[:], JI5[:], S - WIN + JC, JC,
                                    op.min, op.max)
            JOF5 = tp.tile([128, 5], f32)
            nc.vector.tensor_copy(JOF5[:], JI5[:])
            # fy bias: a*jo + (a/2 - s*y)
            BY5 = tp.tile([128, 5], f32)
            nc.vector.tensor_scalar(BY5[:], JOF5[:], A, -A * JC,
                                    op.mult, op.add)
            BXYo = AP(tensor=BXY[:].tensor, offset=BXY[:].offset + 1,
                      ap=[[BXY[:].ap[0][0], 128], [2, 5]])
            nc.vector.tensor_add(BY5[:], BY5[:], BXYo)
            # scatter offsets: h*S*S + jo*S
            OFF5 = tp.tile([128, 5], i32)
            nc.vector.tensor_scalar(OFF5[:], JI5[:], S, S * JC,
                                    op.mult, op.subtract)
            nc.vector.tensor_add(OFF5[:], OFF5[:], KI5[:])
            # fake slots (h >= 544) need no masking: KI5 is exact iota data
            # and JI5 is int-clamped to [JC, S-JC], so their offsets are
            # always >= 544*S*S > the scatter bounds check -> dropped.

            # ---- main loop: one 128-partition group per iteration (the
            # full width keeps the scatter descriptors sprayed across all
            # 16 SDMA engines; narrower scatters collapse onto a few) ----
            for g in range(NG):
                n = 128
                FX = vp.tile([128, S], f16, tag="fx")      # fx row per hm
                nc.scalar.activation(FX[0:n, :], IOTA_I[0:n, :], derf,
                                     bias=BXY[0:n, 2 * g:2 * g + 1], scale=A)
                FY = vp.tile([128, WIN], f16, tag="fy")    # fy row per hm
                nc.scalar.activation(FY[0:n, :], RIOTA[0:n, :], derf,
                                     bias=BY5[0:n, g:g + 1], scale=A)
                nc.vector.tensor_scalar_mul(FY[0:n, :], FY[0:n, :], AMP)

                fyap = FY[0:n, :]
                fxap = FX[0:n, :]
                G = mp.tile([128, FREE], f16, tag="g")
                in0 = AP(tensor=fyap.tensor, offset=fyap.offset,
                         ap=[[fyap.ap[0][0], n], [1, WIN], [0, S]])
                in1 = AP(tensor=fxap.tensor, offset=fxap.offset,
                         ap=[[fxap.ap[0][0], n], [0, WIN], [1, S]])
                nc.vector.tensor_tensor(G[0:n, :], in0, in1, op.mult)
                nc.gpsimd.indirect_dma_start(
                    o2d,
                    IndirectOffsetOnAxis(ap=OFF5[0:n, g:g + 1], axis=0),
                    G[0:n, :], None,
                    bounds_check=OUT_ELEMS - FREE, oob_is_err=False)
                # The scatters write provably-disjoint window blocks (each
                # heatmap owns its 16384-elem range), but Tile can't see
                # that through the dynamic offsets and would serialize them
                # on a WAW dep over 'out'. Clearing the recorded accesses
                # lets the scatters pipeline; kernel-end completion is still
                # enforced through each scatter's G-tile release.
                tc.dep_state.clear_tensor_accesses("out")

    nc.compile()
    return nc


def _get_nc():
    if "nc" not in _cache:
        _cache["nc"] = _build()
    return _cache["nc"]


def _run(coords_full, trace=False):
    from concourse.bass_utils import run_bass_kernel_spmd

    coords_full = np.ascontiguousarray(np.asarray(coords_full, dtype=np.float32))
    assert coords_full.shape == (B_TOTAL, 2 * NUM_CLASS)
    nc = _get_nc()
    in_maps = [{"coords": coords_full[i * B_LOC:(i + 1) * B_LOC]}
               for i in range(N_CORES)]
    br = run_bass_kernel_spmd(nc, in_maps, core_ids=list(range(N_CORES)),
                              trace=trace)
    parts = [br.results[i]["out"].reshape(B_LOC, NUM_CLASS, S, S)
             for i in range(N_CORES)]
    full = np.concatenate(parts, axis=0)
    return full, br


def kernel(coords):
    return _run(coords, trace=False)[0]


# revision 16
# speedup vs baseline: 1.0751x; 1.0252x over previous
"""Coord2HeatmapNet Trainium2 kernel.

out[b,c,j,i] = 10*exp(-(((i+.5)/128 - x)^2 + ((j+.5)/128 - y)^2) / (2*(2/128)^2))

Exploited structure:
  * Separable: each heatmap = fy[j] (x) fx[i] outer product.
  * The value at r pixels from the peak is 10*exp(-r^2/8); beyond ~7 px it is
    under the correctness gate, so only a WIN=14-row window per heatmap is
    materialized; the pre-zeroed output buffer keeps the rest 0.
  * Derivative_Erf activation = 2/sqrt(pi)*exp(-t^2): one ScalarE op per
    gaussian factor vector.
  * Heatmap h = 5p + g lives on partition p of group g, so the per-partition
    coord table is CONTIGUOUS in DRAM: one 40B run per partition, one cheap
    descriptor each -- the completion semaphore fires ~5us earlier than the
    stride-2 gather layout (h = p + 128g) it replaces.
  * Partition p of group g holds the whole 14x128 window of its heatmap as
    1792 contiguous floats. The outer product is one DVE tensor_tensor with
    stride-0 broadcasts; the write-out is ONE indirect scatter DMA per group
    (one offset per partition, 7KB contiguous per heatmap at its
    data-dependent window position).
  * The scatters write provably-disjoint blocks; Tile's conservative WAW dep
    over 'out' is cleared after each scatter so they pipeline behind the DVE.

Sharding: pure data parallel, 8 batches per core across 8 NeuronCores.
"""
import sys

for _p in ("/opt/trn_rl_repo", "/root/.axon_site", "/root/.axon_site/_ro/trn_rl_repo",
           "/root/.axon_site/_ro/pypackages"):
    if _p not in sys.path:
        sys.path.append(_p)

import numpy as np

S = 128
NUM_CLASS = 68
B_TOTAL = 64
N_CORES = 8
B_LOC = B_TOTAL // N_CORES            # 8 batches per core
NHM = B_LOC * NUM_CLASS               # 544 heatmaps per core
WIN = 12                              # window rows per heatmap
NG = 5                                # heatmap h = 5p + g; slots with
                                      # h >= NHM are dropped by the scatter's
                                      # bounds check (96 fake slots)
FREE = WIN * S                        # 1792 elems (7KB) per heatmap window
SIGMA = 2.0 / S
DENOM = 2.0 * SIGMA * SIGMA           # 1/2048
SINV = float(np.sqrt(1.0 / DENOM))    # 45.254834
A = SINV / S
AMP = float(10.0 * np.pi / 4.0)
OUT_ELEMS = NHM * S * S

_cache = {}


def _build():
    import concourse.bass as bass
    import concourse.tile as tile
    from concourse import bacc, mybir
    from concourse.bass import IndirectOffsetOnAxis
    from concourse.bass_types import AP

    f32 = mybir.dt.float32
    f16 = mybir.dt.float16
    i32 = mybir.dt.int32
    nc = bacc.Bacc("TRN2", target_bir_lowering=False, debug=False,
                   num_devices=N_CORES)

    coords = nc.dram_tensor("coords", [B_LOC, 2 * NUM_CLASS], f32,
                            kind="ExternalInput")
    out = nc.dram_tensor("out", [OUT_ELEMS], f32, kind="ExternalOutput")
    o2d = out.ap().rearrange("(a b) -> a b", b=1)
    cflat = coords.ap().rearrange("b f -> (b f)")

    derf = mybir.ActivationFunctionType.Derivative_Erf
    op = mybir.AluOpType
    JC = WIN // 2

    with tile.TileContext(nc) as tc:
        with tc.tile_pool(name="tabs", bufs=1) as tp, \
             tc.tile_pool(name="main", bufs=5) as mp, \
             tc.tile_pool(name="vecs", bufs=2) as vp:
            # ---- coord table: C[p, 2g+h] = coords_flat[10p + 2g + h], i.e.
            # (x, y) of heatmap 5p+g at cols (2g, 2g+1). One contiguous 40B
            # run per partition; partition 108 split off to stay in bounds
            # (it only owns groups 0..3 = 8 elems).
            C = tp.tile([128, 10], f32)
            nc.gpsimd.memset(C[96:128, :], 0.5)
            nc.sync.dma_start(
                C[0:108, :],
                AP(tensor=cflat.tensor, offset=0, ap=[[10, 108], [1, 10]]))
            nc.scalar.dma_start(
                C[108:109, 0:8],
                AP(tensor=cflat.tensor, offset=1080, ap=[[8, 1], [1, 8]]))
            cp = C[:].ap[0][0]          # C partition stride
            Y5 = AP(tensor=C[:].tensor, offset=C[:].offset + 1,
                    ap=[[cp, 128], [2, 5]])

            # iotas (gpsimd) run while the coord DMAs are in flight
            IOTA_I = tp.tile([128, S], f32)
            nc.gpsimd.iota(IOTA_I[:], pattern=[[1, S]], base=0,
                           channel_multiplier=0,
                           allow_small_or_imprecise_dtypes=True)
            RIOTA = tp.tile([128, WIN], f32)
            nc.gpsimd.iota(RIOTA[:], pattern=[[1, WIN]], base=0,
                           channel_multiplier=0,
                           allow_small_or_imprecise_dtypes=True)
            # KI5[p, g] = 5p + g, scaled to the heatmap DRAM base below
            KI5 = tp.tile([128, 5], i32)
            nc.gpsimd.iota(KI5[:], pattern=[[1, 5]], base=0,
                           channel_multiplier=5,
                           allow_small_or_imprecise_dtypes=True)

            # warm-ups: a dep-free activation forces both act-table loads to
            # run back-to-back during the coords DMA instead of waiting on
            # the bias chain; a tiny indirect GATHER (read-only) pays the Q7
            # SWDGE IRAM warmup before the first real scatter.
            warm = tp.tile([128, 1], f32)
            nc.scalar.activation(warm[0:1, :], IOTA_I[0:1, 0:1], derf)
            WOFF = tp.tile([128, 1], i32)
            nc.gpsimd.iota(WOFF[:], pattern=[[1, 1]], base=OUT_ELEMS,
                           channel_multiplier=0,
                           allow_small_or_imprecise_dtypes=True)
            wg = tp.tile([128, 1], f16)
            nc.gpsimd.iota(wg[:], pattern=[[1, 1]], base=0,
                           channel_multiplier=0,
                           allow_small_or_imprecise_dtypes=True)
            nc.gpsimd.indirect_dma_start(
                o2d, IndirectOffsetOnAxis(ap=WOFF[:], axis=0),
                wg[:], None,
                bounds_check=OUT_ELEMS - FREE, oob_is_err=False)
            tc.dep_state.clear_tensor_accesses("out")

            # ---- bulk table math ----
            # BXY = a/2 - s*coord: fx bias at even cols, fy bias (minus the
            # a*jo term added below) at odd cols
            BXY = tp.tile([128, 10], f32)
            nc.vector.tensor_scalar(BXY[:], C[:], -SINV, A * 0.5,
                                    op.mult, op.add)
            # JOF5 = jo + WIN/2 = clamp(rint(128y), WIN/2, S-WIN+WIN/2),
            # rounded in float via the +2^23 trick; the -WIN/2 is folded
            # into the BY5 / OFF5 constants. (C's garbage partitions are
            # memset, so the float clamp never sees NaN.)
            M23 = 8388608.0
            JOF5 = tp.tile([128, 5], f32)
            nc.vector.tensor_scalar(JOF5[:], Y5, float(S), M23,
                                    op.mult, op.add)
            nc.vector.tensor_scalar(JOF5[:], JOF5[:], M23, float(S - WIN + JC),
                                    op.subtract, op.min)
            nc.vector.tensor_scalar_max(JOF5[:], JOF5[:], float(JC))
            # fy bias: a*jo + (a/2 - s*y)
            BY5 = tp.tile([128, 5], f32)
            nc.vector.tensor_scalar(BY5[:], JOF5[:], A, -A * JC,
                                    op.mult, op.add)
            BXYo = AP(tensor=BXY[:].tensor, offset=BXY[:].offset + 1,
                      ap=[[BXY[:].ap[0][0], 128], [2, 5]])
            nc.vector.tensor_add(BY5[:], BY5[:], BXYo)
            # scatter offsets: h*S*S + jo*S (off the TT1-critical prefix)
            nc.vector.tensor_scalar_mul(KI5[:], KI5[:], S * S)
            JI5 = tp.tile([128, 5], i32)
            nc.vector.tensor_copy(JI5[:], JOF5[:])
            OFF5 = tp.tile([128, 5], i32)
            nc.vector.tensor_scalar(OFF5[:], JI5[:], S, S * JC,
                                    op.mult, op.subtract)
            nc.vector.tensor_add(OFF5[:], OFF5[:], KI5[:])
            # fake slots (h >= 544) need no masking: KI5 is exact iota data
            # and JI5 is int-clamped to [JC, S-JC], so their offsets are
            # always >= 544*S*S > the scatter bounds check -> dropped.

            # ---- main loop: one 128-partition group per iteration (the
            # full width keeps the scatter descriptors sprayed across all
            # 16 SDMA engines; narrower scatters collapse onto a few) ----
            for g in range(NG):
                n = 128
                FX = vp.tile([128, S], f16, tag="fx")      # fx row per hm
                nc.scalar.activation(FX[0:n, :], IOTA_I[0:n, :], derf,
                                     bias=BXY[0:n, 2 * g:2 * g + 1], scale=A)
                nc.vector.tensor_scalar_mul(FX[0:n, :], FX[0:n, :], AMP)
                FY = vp.tile([128, WIN], f16, tag="fy")    # fy row per hm
                nc.scalar.activation(FY[0:n, :], RIOTA[0:n, :], derf,
                                     bias=BY5[0:n, g:g + 1], scale=A)

                fyap = FY[0:n, :]
                fxap = FX[0:n, :]
                G = mp.tile([128, FREE], f16, tag="g")
                in0 = AP(tensor=fyap.tensor, offset=fyap.offset,
                         ap=[[fyap.ap[0][0], n], [1, WIN], [0, S]])
                in1 = AP(tensor=fxap.tensor, offset=fxap.offset,
                         ap=[[fxap.ap[0][0], n], [0, WIN], [1, S]])
                nc.vector.tensor_tensor(G[0:n, :], in0, in1, op.mult)
                nc.gpsimd.indirect_dma_start(
                    o2d,
                    IndirectOffsetOnAxis(ap=OFF5[0:n, g:g + 1], axis=0),
                    G[0:n, :], None,
                    bounds_check=OUT_ELEMS - FREE, oob_is_err=False)
                # The scatters write provably-disjoint window blocks (each
                # heatmap owns its 16384-elem range), but Tile can't see
                # that through the dynamic offsets and would serialize them
                # on a WAW dep over 'out'. Clearing the recorded accesses
                # lets the scatters pipeline; kernel-end completion is still
                # enforced through each scatter's G-tile release.
                tc.dep_state.clear_tensor_accesses("out")

    nc.compile()
    return nc


def _get_nc():
    if "nc" not in _cache:
        _cache["nc"] = _build()
    return _cache["nc"]


def _run(coords_full, trace=False):
    from concourse.bass_utils import run_bass_kernel_spmd

    coords_full = np.ascontiguousarray(np.asarray(coords_full, dtype=np.float32))
    assert coords_full.shape == (B_TOTAL, 2 * NUM_CLASS)
    nc = _get_nc()
    in_maps = [{"coords": coords_full[i * B_LOC:(i + 1) * B_LOC]}
               for i in range(N_CORES)]
    br = run_bass_kernel_spmd(nc, in_maps, core_ids=list(range(N_CORES)),
                              trace=trace)
    parts = [br.results[i]["out"].reshape(B_LOC, NUM_CLASS, S, S)
             for i in range(N_CORES)]
    full = np.concatenate(parts, axis=0)
    return full, br


def kernel(coords):
    return _run(coords, trace=False)[0]


# revision 17
# speedup vs baseline: 1.0841x; 1.0084x over previous
"""Coord2HeatmapNet Trainium2 kernel.

out[b,c,j,i] = 10*exp(-(((i+.5)/128 - x)^2 + ((j+.5)/128 - y)^2) / (2*(2/128)^2))

Exploited structure:
  * Separable: each heatmap = fy[j] (x) fx[i] outer product.
  * The value at r pixels from the peak is 10*exp(-r^2/8); beyond ~7 px it is
    under the correctness gate, so only a WIN=14-row window per heatmap is
    materialized; the pre-zeroed output buffer keeps the rest 0.
  * Derivative_Erf activation = 2/sqrt(pi)*exp(-t^2): one ScalarE op per
    gaussian factor vector.
  * Heatmap h = 5p + g lives on partition p of group g, so the per-partition
    coord table is CONTIGUOUS in DRAM: one 40B run per partition, one cheap
    descriptor each -- the completion semaphore fires ~5us earlier than the
    stride-2 gather layout (h = p + 128g) it replaces.
  * Partition p of group g holds the whole 14x128 window of its heatmap as
    1792 contiguous floats. The outer product is one DVE tensor_tensor with
    stride-0 broadcasts; the write-out is ONE indirect scatter DMA per group
    (one offset per partition, 7KB contiguous per heatmap at its
    data-dependent window position).
  * The scatters write provably-disjoint blocks; Tile's conservative WAW dep
    over 'out' is cleared after each scatter so they pipeline behind the DVE.

Sharding: pure data parallel, 8 batches per core across 8 NeuronCores.
"""
import sys

for _p in ("/opt/trn_rl_repo", "/root/.axon_site", "/root/.axon_site/_ro/trn_rl_repo",
           "/root/.axon_site/_ro/pypackages"):
    if _p not in sys.path:
        sys.path.append(_p)

import numpy as np

S = 128
NUM_CLASS = 68
B_TOTAL = 64
N_CORES = 8
B_LOC = B_TOTAL // N_CORES            # 8 batches per core
NHM = B_LOC * NUM_CLASS               # 544 heatmaps per core
WIN = 12                              # window rows per heatmap
NG = 5                                # heatmap h = 5p + g; slots with
                                      # h >= NHM are dropped by the scatter's
                                      # bounds check (96 fake slots)
FREE = WIN * S                        # 1792 elems (7KB) per heatmap window
SIGMA = 2.0 / S
DENOM = 2.0 * SIGMA * SIGMA           # 1/2048
SINV = float(np.sqrt(1.0 / DENOM))    # 45.254834
A = SINV / S
AMP = float(10.0 * np.pi / 4.0)
OUT_ELEMS = NHM * S * S

_cache = {}


def _build():
    import concourse.bass as bass
    import concourse.tile as tile
    from concourse import bacc, mybir
    from concourse.bass import IndirectOffsetOnAxis
    from concourse.bass_types import AP

    f32 = mybir.dt.float32
    f16 = mybir.dt.float16
    i32 = mybir.dt.int32
    nc = bacc.Bacc("TRN2", target_bir_lowering=False, debug=False,
                   num_devices=N_CORES)

    coords = nc.dram_tensor("coords", [B_LOC, 2 * NUM_CLASS], f32,
                            kind="ExternalInput")
    out = nc.dram_tensor("out", [OUT_ELEMS], f32, kind="ExternalOutput")
    wscratch = nc.dram_tensor("wscratch", [128], f32, kind="Internal")
    o2d = out.ap().rearrange("(a b) -> a b", b=1)
    cflat = coords.ap().rearrange("b f -> (b f)")

    derf = mybir.ActivationFunctionType.Derivative_Erf
    op = mybir.AluOpType
    JC = WIN // 2

    with tile.TileContext(nc) as tc:
        with tc.tile_pool(name="tabs", bufs=1) as tp, \
             tc.tile_pool(name="main", bufs=5) as mp, \
             tc.tile_pool(name="vecs", bufs=2) as vp:
            # ---- coord table: C[p, 2g+h] = coords_flat[10p + 2g + h], i.e.
            # (x, y) of heatmap 5p+g at cols (2g, 2g+1). One contiguous 40B
            # run per partition; partition 108 split off to stay in bounds
            # (it only owns groups 0..3 = 8 elems).
            C = tp.tile([128, 10], f32)
            nc.gpsimd.memset(C[96:128, :], 0.5)
            nc.sync.dma_start(
                C[0:108, :],
                AP(tensor=cflat.tensor, offset=0, ap=[[10, 108], [1, 10]]))
            nc.scalar.dma_start(
                C[108:109, 0:8],
                AP(tensor=cflat.tensor, offset=1080, ap=[[8, 1], [1, 8]]))
            cp = C[:].ap[0][0]          # C partition stride
            Y5 = AP(tensor=C[:].tensor, offset=C[:].offset + 1,
                    ap=[[cp, 128], [2, 5]])

            # iotas (gpsimd) run while the coord DMAs are in flight
            IOTA_I = tp.tile([128, S], f32)
            nc.gpsimd.iota(IOTA_I[:], pattern=[[1, S]], base=0,
                           channel_multiplier=0,
                           allow_small_or_imprecise_dtypes=True)
            RIOTA = tp.tile([128, WIN], f32)
            nc.gpsimd.iota(RIOTA[:], pattern=[[1, WIN]], base=0,
                           channel_multiplier=0,
                           allow_small_or_imprecise_dtypes=True)
            # KI5[p, g] = 5p + g, scaled to the heatmap DRAM base below
            KI5 = tp.tile([128, 5], i32)
            nc.gpsimd.iota(KI5[:], pattern=[[1, 5]], base=0,
                           channel_multiplier=5,
                           allow_small_or_imprecise_dtypes=True)

            # warm-ups: a dep-free activation forces both act-table loads to
            # run back-to-back during the coords DMA instead of waiting on
            # the bias chain; a tiny indirect GATHER (read-only) pays the Q7
            # SWDGE IRAM warmup before the first real scatter.
            warm = tp.tile([128, 1], f32)
            nc.scalar.activation(warm[0:1, :], IOTA_I[0:1, 0:1], derf)
            WOFF = tp.tile([128, 1], i32)
            nc.gpsimd.iota(WOFF[:], pattern=[[1, 1]], base=0,
                           channel_multiplier=1,
                           allow_small_or_imprecise_dtypes=True)
            wg = tp.tile([128, 1], f16)
            nc.gpsimd.iota(wg[:], pattern=[[1, 1]], base=0,
                           channel_multiplier=0,
                           allow_small_or_imprecise_dtypes=True)
            nc.gpsimd.indirect_dma_start(
                wscratch.ap().rearrange("(a b) -> a b", b=1),
                IndirectOffsetOnAxis(ap=WOFF[:], axis=0),
                wg[:], None)

            # ---- bulk table math ----
            # BXY = a/2 - s*coord: fx bias at even cols, fy bias (minus the
            # a*jo term added below) at odd cols
            BXY = tp.tile([128, 10], f32)
            nc.vector.tensor_scalar(BXY[:], C[:], -SINV, A * 0.5,
                                    op.mult, op.add)
            # JOF5 = jo + WIN/2 = clamp(rint(128y), WIN/2, S-WIN+WIN/2),
            # rounded in float via the +2^23 trick; the -WIN/2 is folded
            # into the BY5 / OFF5 constants. (C's garbage partitions are
            # memset, so the float clamp never sees NaN.)
            M23 = 8388608.0
            JOF5 = tp.tile([128, 5], f32)
            nc.vector.tensor_scalar(JOF5[:], Y5, float(S), M23,
                                    op.mult, op.add)
            nc.vector.tensor_scalar(JOF5[:], JOF5[:], M23, float(S - WIN + JC),
                                    op.subtract, op.min)
            nc.vector.tensor_scalar_max(JOF5[:], JOF5[:], float(JC))
            # fy bias: a*jo + (a/2 - s*y)
            BY5 = tp.tile([128, 5], f32)
            nc.vector.tensor_scalar(BY5[:], JOF5[:], A, -A * JC,
                                    op.mult, op.add)
            BXYo = AP(tensor=BXY[:].tensor, offset=BXY[:].offset + 1,
                      ap=[[BXY[:].ap[0][0], 128], [2, 5]])
            nc.vector.tensor_add(BY5[:], BY5[:], BXYo)
            # scatter offsets: h*S*S + jo*S (off the TT1-critical prefix)
            nc.vector.tensor_scalar_mul(KI5[:], KI5[:], S * S)
            JI5 = tp.tile([128, 5], i32)
            nc.vector.tensor_copy(JI5[:], JOF5[:])
            OFF5 = tp.tile([128, 5], i32)
            nc.vector.tensor_scalar(OFF5[:], JI5[:], S, S * JC,
                                    op.mult, op.subtract)
            nc.vector.tensor_add(OFF5[:], OFF5[:], KI5[:])
            # fake slots (h >= 544) need no masking: KI5 is exact iota data
            # and JI5 is int-clamped to [JC, S-JC], so their offsets are
            # always >= 544*S*S > the scatter bounds check -> dropped.

            # ---- main loop: one 128-partition group per iteration (the
            # full width keeps the scatter descriptors sprayed across all
            # 16 SDMA engines; narrower scatters collapse onto a few) ----
            for g in range(NG):
                n = 128
                FX = vp.tile([128, S], f16, tag="fx")      # fx row per hm
                nc.scalar.activation(FX[0:n, :], IOTA_I[0:n, :], derf,
                                     bias=BXY[0:n, 2 * g:2 * g + 1], scale=A)
                nc.vector.tensor_scalar_mul(FX[0:n, :], FX[0:n, :], AMP)
                FY = vp.tile([128, WIN], f16, tag="fy")    # fy row per hm
                nc.scalar.activation(FY[0:n, :], RIOTA[0:n, :], derf,
                                     bias=BY5[0:n, g:g + 1], scale=A)

                fyap = FY[0:n, :]
                fxap = FX[0:n, :]
                G = mp.tile([128, FREE], f16, tag="g")
                in0 = AP(tensor=fyap.tensor, offset=fyap.offset,
                         ap=[[fyap.ap[0][0], n], [1, WIN], [0, S]])
                in1 = AP(tensor=fxap.tensor, offset=fxap.offset,
                         ap=[[fxap.ap[0][0], n], [0, WIN], [1, S]])
                nc.vector.tensor_tensor(G[0:n, :], in0, in1, op.mult)
                nc.gpsimd.indirect_dma_start(
                    o2d,
                    IndirectOffsetOnAxis(ap=OFF5[0:n, g:g + 1], axis=0),
                    G[0:n, :], None,
                    bounds_check=OUT_ELEMS - FREE, oob_is_err=False)
                # The scatters write provably-disjoint window blocks (each
                # heatmap owns its 16384-elem range), but Tile can't see
                # that through the dynamic offsets and would serialize them
                # on a WAW dep over 'out'. Clearing the recorded accesses
                # lets the scatters pipeline; kernel-end completion is still
                # enforced through each scatter's G-tile release.
                tc.dep_state.clear_tensor_accesses("out")

    nc.compile()
    return nc


def _get_nc():
    if "nc" not in _cache:
        _cache["nc"] = _build()
    return _cache["nc"]


def _run(coords_full, trace=False):
    from concourse.bass_utils import run_bass_kernel_spmd

    coords_full = np.ascontiguousarray(np.asarray(coords_full, dtype=np.float32))
    assert coords_full.shape == (B_TOTAL, 2 * NUM_CLASS)
    nc = _get_nc()
    in_maps = [{"coords": coords_full[i * B_LOC:(i + 1) * B_LOC]}
               for i in range(N_CORES)]
    br = run_bass_kernel_spmd(nc, in_maps, core_ids=list(range(N_CORES)),
                              trace=trace)
    parts = [br.results[i]["out"].reshape(B_LOC, NUM_CLASS, S, S)
             for i in range(N_CORES)]
    full = np.concatenate(parts, axis=0)
    return full, br


def kernel(coords):
    return _run(coords, trace=False)[0]
